# revision 26
# baseline (speedup 1.0000x reference)
"""Trainium2 Bass kernel for the Binde-ESN MNIST model.

Math per batch row b (T=28 steps, MID=128):
    pre1_t = x_t @ w_in + x1_{t-1} @ m1 + x2_{t-1} @ m21 + bias1
    x1_t   = tanh(pre1_t)
    x2_t   = tanh(pre1_t @ m12 + x2_{t-1} @ m2 + bias2)
    output = x2_T @ fc_w + fc_b
with m1 = w_res1*binde1, m21 = w_res21*binde2, m12 = w_res12*binde3,
m2 = w_res2*binde4, bias1 = b_in+b_x1+b_res21, bias2 = b_res12+b_x2.

Distribution: pure data parallel across 8 NeuronCores (4096 batch rows
per core); the 128x128 weights are replicated. On-device layout keeps
features on SBUF partitions and batch on the free dim, so every matmul
is stationary-weight x moving-activations. Matmuls run in float32r
(full-rate fp32 on the PE at N>=256, ~1e-4 rounding), accumulation is
fp32 in PSUM, tanh runs on the scalar engine with its free per-partition
bias, and the pre-activation copy+bias runs on the vector engine.
"""

import time

import numpy as np

import concourse.mybir as mybir
import concourse.tile as tile
from concourse import bacc
from concourse.bass_utils import run_bass_kernel_spmd

F32 = mybir.dt.float32
F32R = mybir.dt.float32r
AF = mybir.ActivationFunctionType

B = 32768
T = 28
F = 28
MID = 128
FP = 128               # F padded to full contraction (partial-K matmuls run at half rate)
NCORES = 8
BC = B // NCORES          # batch per core
CHUNK = 1024              # batch columns per PSUM tile (2 banks)
NCHUNK = BC // CHUNK
NSLICE = CHUNK // 512     # matmul N<=512 fp32 sub-slices per chunk

_cached = {}


def _build():
    nc = bacc.Bacc("TRN2", target_bir_lowering=False, debug=False,
                   num_devices=NCORES)

    xT = nc.declare_dram_parameter("xT", [T, FP, BC], F32R, isOutput=False)
    w_in = nc.declare_dram_parameter("w_in", [FP, MID], F32R, isOutput=False)
    m1 = nc.declare_dram_parameter("m1", [MID, MID], F32R, isOutput=False)
    m21 = nc.declare_dram_parameter("m21", [MID, MID], F32R, isOutput=False)
    m12 = nc.declare_dram_parameter("m12", [MID, MID], F32R, isOutput=False)
    m2 = nc.declare_dram_parameter("m2", [MID, MID], F32R, isOutput=False)
    fc_w = nc.declare_dram_parameter("fc_w", [MID, 10], F32R, isOutput=False)
    bias1 = nc.declare_dram_parameter("bias1", [MID, 1], F32, isOutput=False)
    bias2 = nc.declare_dram_parameter("bias2", [MID, 1], F32, isOutput=False)
    fc_b = nc.declare_dram_parameter("fc_b", [10, 1], F32, isOutput=False)

    x1T = nc.declare_dram_parameter("x1T", [MID, BC], F32R, isOutput=True)
    x2T = nc.declare_dram_parameter("x2T", [MID, BC], F32R, isOutput=True)
    outT = nc.declare_dram_parameter("outT", [10, BC], F32, isOutput=True)

    with tile.TileContext(nc) as tc:
        with (
            tc.tile_pool(name="const", bufs=1) as cpool,
            tc.tile_pool(name="state", bufs=1) as spool,
            tc.tile_pool(name="xin", bufs=4) as xpool,
            tc.tile_pool(name="ps1", bufs=4, space="PSUM") as ps1pool,
            tc.tile_pool(name="ps2", bufs=2, space="PSUM") as ps2pool,
        ):

            # warmup: dependency-free matmuls on a memset tile keep the PE
            # busy through the input DMAs so the HAM clock gate is released
            # (2.4 GHz) by the time the real matmuls start
            warm = cpool.tile([MID, 128], mybir.dt.bfloat16)
            nc.vector.memset(warm[:], 0.0)
            warm_ps = ps1pool.tile([MID, 512], F32, name="warm_ps",
                                   tag="ps1t")
            for _ in range(30):
                nc.tensor.matmul(warm_ps[:, 0:128], warm[:], warm[:],
                                 start=True, stop=True)

            # first matmul needs only w_in + the first x chunk: order the
            # startup DMAs so those land first on the queue
            w_in_t = cpool.tile([FP, MID], F32R)
            nc.sync.dma_start(w_in_t[:], w_in[:])
            bias1_t = cpool.tile([MID, 1], F32)
            nc.sync.dma_start(bias1_t[:], bias1[:])
            xt_first = xpool.tile([FP, BC], F32R, name="xt", tag="xt")
            for c in range(NCHUNK):
                nc.sync.dma_start(xt_first[:, c * CHUNK:(c + 1) * CHUNK],
                                  xT[0][:, c * CHUNK:(c + 1) * CHUNK])
            m12_t = cpool.tile([MID, MID], F32R)
            nc.sync.dma_start(m12_t[:], m12[:])
            bias2_t = cpool.tile([MID, 1], F32)
            nc.sync.dma_start(bias2_t[:], bias2[:])
            m1_t = cpool.tile([MID, MID], F32R)
            nc.sync.dma_start(m1_t[:], m1[:])
            m21_t = cpool.tile([MID, MID], F32R)
            nc.sync.dma_start(m21_t[:], m21[:])
            m2_t = cpool.tile([MID, MID], F32R)
            nc.sync.dma_start(m2_t[:], m2[:])
            fc_w_t = cpool.tile([MID, 10], F32R)
            nc.sync.dma_start(fc_w_t[:], fc_w[:])
            fc_b_t = cpool.tile([10, 1], F32)
            nc.sync.dma_start(fc_b_t[:], fc_b[:])

            # states and pre live as 2048-wide pair tiles so tanh runs as
            # wide ACT ops; matmuls read 512-column slices (region deps)
            PAIR = 2 * CHUNK
            x1_p = [spool.tile([MID, PAIR], F32R, name=f"x1_{p}")
                    for p in range(NCHUNK // 2)]
            x2_p = [spool.tile([MID, PAIR], F32R, name=f"x2_{p}")
                    for p in range(NCHUNK // 2)]
            pre_p = [spool.tile([MID, PAIR], F32R, name=f"pre_{p}")
                     for p in range(NCHUNK // 2)]

            def xsl(c, s):
                return slice(c * CHUNK + 512 * s, c * CHUNK + 512 * (s + 1))

            def psl(c, s):
                return slice((c % 2) * CHUNK + 512 * s,
                             (c % 2) * CHUNK + 512 * (s + 1))

            def emit_ps1(t, xt, c):
                # single-bank ps1 tiles: the pre copy for each 512 slice
                # starts right after its 3-matmul group stops
                group1 = [(w_in_t, lambda s: xt[:, xsl(c, s)])]
                if t > 0:
                    group1.append((m1_t, lambda s: x1_p[c // 2][:, psl(c, s)]))
                    group1.append((m21_t, lambda s: x2_p[c // 2][:, psl(c, s)]))
                for s in range(NSLICE):
                    ps1 = ps1pool.tile([MID, 512], F32, name="ps1t",
                                       tag="ps1t")
                    for wi, (w_t, mov) in enumerate(group1):
                        nc.tensor.matmul(
                            ps1[:], w_t[:], mov(s),
                            start=(wi == 0), stop=(wi == len(group1) - 1))
                    nc.vector.tensor_scalar_add(
                        pre_p[c // 2][:, psl(c, s)], ps1[:], bias1_t[:, 0:1])

            def emit_ps2(t, c):
                ps2 = ps2pool.tile([MID, CHUNK], F32, name="ps2t", tag="ps2t")
                group2 = [(m12_t, lambda s: pre_p[c // 2][:, psl(c, s)])]
                if t > 0:
                    group2.append((m2_t, lambda s: x2_p[c // 2][:, psl(c, s)]))
                for wi, (w_t, mov) in enumerate(group2):
                    for s in range(NSLICE):
                        nc.tensor.matmul(
                            ps2[:, 512 * s:512 * (s + 1)], w_t[:], mov(s),
                            start=(wi == 0), stop=(wi == len(group2) - 1))
                # x2 state: tanh(psum + bias2), written into the pair tile
                nc.scalar.activation(
                    x2_p[c // 2][:, (c % 2) * CHUNK:(c % 2 + 1) * CHUNK],
                    ps2[:], AF.Tanh, bias=bias2_t[:, 0:1])

            for t in range(T):
                if t == 0:
                    xt = xt_first
                else:
                    xt = xpool.tile([FP, BC], F32R, name="xt", tag="xt")
                    for c in range(NCHUNK):
                        nc.sync.dma_start(xt[:, c * CHUNK:(c + 1) * CHUNK],
                                          xT[t][:, c * CHUNK:(c + 1) * CHUNK])
                # pipelined chunk schedule; tanh(x1) runs 2048-wide on the
                # SBUF pre pair once both halves are written
                emit_ps1(t, xt, 0)
                if t == 0:
                    wps = ps2pool.tile([MID, CHUNK], F32, name="ps2t",
                                       tag="ps2t")
                    for _ in range(10):
                        nc.tensor.matmul(wps[:, 0:128], warm[:], warm[:],
                                         start=True, stop=True)
                emit_ps1(t, xt, 1)
                if t == 0:
                    wps = ps2pool.tile([MID, CHUNK], F32, name="ps2t",
                                       tag="ps2t")
                    for _ in range(10):
                        nc.tensor.matmul(wps[:, 0:128], warm[:], warm[:],
                                         start=True, stop=True)
                emit_ps1(t, xt, 2)
                nc.scalar.activation(x1_p[0][:], pre_p[0][:], AF.Tanh)
                emit_ps2(t, 0)
                emit_ps1(t, xt, 3)
                emit_ps2(t, 1)
                nc.scalar.activation(x1_p[1][:], pre_p[1][:], AF.Tanh)
                emit_ps2(t, 2)
                emit_ps2(t, 3)

            # classifier + final-state DMAs, per pair as soon as the
            # last-step tanh for that pair lands
            out_sb = cpool.tile([10, BC], F32)
            for p in range(NCHUNK // 2):
                c0 = p * PAIR
                nc.sync.dma_start(x1T[:, c0:c0 + PAIR], x1_p[p][:])
                nc.sync.dma_start(x2T[:, c0:c0 + PAIR], x2_p[p][:])
                for c in (2 * p, 2 * p + 1):
                    cc = c * CHUNK
                    for s in range(NSLICE):
                        psf = ps1pool.tile([MID, 512], F32, name="psf",
                                           tag="ps1t")
                        nc.tensor.matmul(
                            psf[0:10, :], fc_w_t[:],
                            x2_p[p][:, psl(c, s)],
                            start=True, stop=True)
                        nc.scalar.activation(
                            out_sb[:, cc + 512 * s:cc + 512 * (s + 1)],
                            psf[0:10, :],
                            AF.Identity, bias=fc_b_t[:, 0:1])
                    nc.sync.dma_start(outT[:, cc:cc + CHUNK],
                                      out_sb[:, cc:cc + CHUNK])

    nc.compile()
    return nc


def kernel(x, binde1, binde2, binde3, binde4,
           w_in, w_res1, w_res12, w_res2, w_res21,
           b_in, b_x1, b_res12, b_x2, b_res21, fc_w, fc_b):
    if "nc" not in _cached:
        _cached["nc"] = _build()
    nc = _cached["nc"]

    x = np.asarray(x, dtype=np.float32)
    xT = np.zeros((T, FP, B), dtype=np.float32)
    xT[:, :F, :] = x.reshape(B, T, F).transpose(1, 2, 0)

    m1 = np.asarray(w_res1 * binde1, dtype=np.float32)
    m21 = np.asarray(w_res21 * binde2, dtype=np.float32)
    m12 = np.asarray(w_res12 * binde3, dtype=np.float32)
    m2 = np.asarray(w_res2 * binde4, dtype=np.float32)
    bias1 = np.asarray(b_in + b_x1 + b_res21, dtype=np.float32).reshape(MID, 1)
    bias2 = np.asarray(b_res12 + b_x2, dtype=np.float32).reshape(MID, 1)
    w_in_np = np.zeros((FP, MID), dtype=np.float32)
    w_in_np[:F] = np.asarray(w_in, dtype=np.float32)
    fc_w_np = np.asarray(fc_w, dtype=np.float32)
    fc_b_np = np.asarray(fc_b, dtype=np.float32).reshape(10, 1)

    shared = dict(w_in=w_in_np, m1=m1, m21=m21, m12=m12, m2=m2,
                  fc_w=fc_w_np, bias1=bias1, bias2=bias2, fc_b=fc_b_np)
    in_maps = []
    for i in range(NCORES):
        xs = np.ascontiguousarray(xT[:, :, i * BC:(i + 1) * BC])
        in_maps.append(dict(xT=xs, **shared))

    # the device occasionally throws a transient unrecoverable-exec-unit
    # error; a fresh run recovers it
    last_err = None
    for attempt in range(3):
        try:
            res = run_bass_kernel_spmd(nc, in_maps, list(range(NCORES)))
            break
        except Exception as e:  # noqa: BLE001
            last_err = e
            time.sleep(2.0)
    else:
        raise last_err

    output = np.empty((B, 10), dtype=np.float32)
    x1 = np.empty((B, MID), dtype=np.float32)
    x2 = np.empty((B, MID), dtype=np.float32)
    for i in range(NCORES):
        sl = slice(i * BC, (i + 1) * BC)
        output[sl] = res.results[i]["outT"].T
        x1[sl] = res.results[i]["x1T"].T
        x2[sl] = res.results[i]["x2T"].T
    return output, x1, x2


# revision 27
# speedup vs baseline: 1.0464x; 1.0464x over previous
"""Trainium2 Bass kernel for the Binde-ESN MNIST model.

Math per batch row b (T=28 steps, MID=128):
    pre1_t = x_t @ w_in + x1_{t-1} @ m1 + x2_{t-1} @ m21 + bias1
    x1_t   = tanh(pre1_t)
    x2_t   = tanh(pre1_t @ m12 + x2_{t-1} @ m2 + bias2)
    output = x2_T @ fc_w + fc_b
with m1 = w_res1*binde1, m21 = w_res21*binde2, m12 = w_res12*binde3,
m2 = w_res2*binde4, bias1 = b_in+b_x1+b_res21, bias2 = b_res12+b_x2.

Distribution: pure data parallel across 8 NeuronCores (4096 batch rows
per core); the 128x128 weights are replicated. On-device layout keeps
features on SBUF partitions and batch on the free dim, so every matmul
is stationary-weight x moving-activations. Matmuls run in float32r
(full-rate fp32 on the PE at N>=256, ~1e-4 rounding), accumulation is
fp32 in PSUM, tanh runs on the scalar engine with its free per-partition
bias, and the pre-activation copy+bias runs on the vector engine.
"""

import time

import numpy as np

import concourse.mybir as mybir
import concourse.tile as tile
from concourse import bacc
from concourse.bass_utils import run_bass_kernel_spmd

F32 = mybir.dt.float32
F32R = mybir.dt.float32r
AF = mybir.ActivationFunctionType

B = 32768
T = 28
F = 28
MID = 128
FP = 128               # F padded to full contraction (partial-K matmuls run at half rate)
NCORES = 8
BC = B // NCORES          # batch per core
CHUNK = 1024              # batch columns per PSUM tile (2 banks)
NCHUNK = BC // CHUNK
NSLICE = CHUNK // 512     # matmul N<=512 fp32 sub-slices per chunk

_cached = {}


def _build():
    nc = bacc.Bacc("TRN2", target_bir_lowering=False, debug=False,
                   num_devices=NCORES)

    xT = nc.declare_dram_parameter("xT", [T, FP, BC], F32R, isOutput=False)
    w_in = nc.declare_dram_parameter("w_in", [FP, MID], F32R, isOutput=False)
    m1 = nc.declare_dram_parameter("m1", [MID, MID], F32R, isOutput=False)
    m21 = nc.declare_dram_parameter("m21", [MID, MID], F32R, isOutput=False)
    m12 = nc.declare_dram_parameter("m12", [MID, MID], F32R, isOutput=False)
    m2 = nc.declare_dram_parameter("m2", [MID, MID], F32R, isOutput=False)
    fc_w = nc.declare_dram_parameter("fc_w", [MID, 10], F32R, isOutput=False)
    bias1 = nc.declare_dram_parameter("bias1", [MID, 1], F32, isOutput=False)
    bias2 = nc.declare_dram_parameter("bias2", [MID, 1], F32, isOutput=False)
    fc_b = nc.declare_dram_parameter("fc_b", [10, 1], F32, isOutput=False)

    x1T = nc.declare_dram_parameter("x1T", [MID, BC], F32R, isOutput=True)
    x2T = nc.declare_dram_parameter("x2T", [MID, BC], F32R, isOutput=True)
    outT = nc.declare_dram_parameter("outT", [10, BC], F32, isOutput=True)

    with tile.TileContext(nc) as tc:
        with (
            tc.tile_pool(name="const", bufs=1) as cpool,
            tc.tile_pool(name="state", bufs=1) as spool,
            tc.tile_pool(name="xin", bufs=4) as xpool,
            tc.tile_pool(name="ps1", bufs=4, space="PSUM") as ps1pool,
            tc.tile_pool(name="ps2", bufs=2, space="PSUM") as ps2pool,
        ):

            # warmup: dependency-free matmuls on a memset tile keep the PE
            # busy through the input DMAs so the HAM clock gate is released
            # (2.4 GHz) by the time the real matmuls start
            warm = cpool.tile([MID, 128], mybir.dt.bfloat16)
            nc.vector.memset(warm[:], 0.0)
            warm_ps = ps1pool.tile([MID, 512], F32, name="warm_ps",
                                   tag="ps1t")
            for _ in range(30):
                nc.tensor.matmul(warm_ps[:, 0:128], warm[:], warm[:],
                                 start=True, stop=True)

            # first matmul needs only w_in + the first x chunk: order the
            # startup DMAs so those land first on the queue
            w_in_t = cpool.tile([FP, MID], F32R)
            nc.sync.dma_start(w_in_t[:], w_in[:])
            bias1_t = cpool.tile([MID, 1], F32)
            nc.sync.dma_start(bias1_t[:], bias1[:])
            xt_first = xpool.tile([FP, BC], F32R, name="xt", tag="xt")
            for c in range(NCHUNK):
                nc.sync.dma_start(xt_first[:, c * CHUNK:(c + 1) * CHUNK],
                                  xT[0][:, c * CHUNK:(c + 1) * CHUNK])
            m12_t = cpool.tile([MID, MID], F32R)
            nc.sync.dma_start(m12_t[:], m12[:])
            bias2_t = cpool.tile([MID, 1], F32)
            nc.sync.dma_start(bias2_t[:], bias2[:])
            m1_t = cpool.tile([MID, MID], F32R)
            nc.sync.dma_start(m1_t[:], m1[:])
            m21_t = cpool.tile([MID, MID], F32R)
            nc.sync.dma_start(m21_t[:], m21[:])
            m2_t = cpool.tile([MID, MID], F32R)
            nc.sync.dma_start(m2_t[:], m2[:])
            fc_w_t = cpool.tile([MID, 10], F32R)
            nc.sync.dma_start(fc_w_t[:], fc_w[:])
            fc_b_t = cpool.tile([10, 1], F32)
            nc.sync.dma_start(fc_b_t[:], fc_b[:])

            # states and pre live as 2048-wide pair tiles so tanh runs as
            # wide ACT ops; matmuls read 512-column slices (region deps)
            PAIR = 2 * CHUNK
            x1_p = [spool.tile([MID, PAIR], F32R, name=f"x1_{p}")
                    for p in range(NCHUNK // 2)]
            x2_p = [spool.tile([MID, PAIR], F32R, name=f"x2_{p}")
                    for p in range(NCHUNK // 2)]
            pre_p = [spool.tile([MID, PAIR], F32R, name=f"pre_{p}")
                     for p in range(NCHUNK // 2)]

            def xsl(c, s):
                return slice(c * CHUNK + 512 * s, c * CHUNK + 512 * (s + 1))

            def psl(c, s):
                return slice((c % 2) * CHUNK + 512 * s,
                             (c % 2) * CHUNK + 512 * (s + 1))

            def emit_ps1(t, xt, c):
                # single-bank ps1 tiles: the pre copy for each 512 slice
                # starts right after its 3-matmul group stops
                group1 = [(w_in_t, lambda s: xt[:, xsl(c, s)])]
                if t > 0:
                    group1.append((m1_t, lambda s: x1_p[c // 2][:, psl(c, s)]))
                    group1.append((m21_t, lambda s: x2_p[c // 2][:, psl(c, s)]))
                for s in range(NSLICE):
                    ps1 = ps1pool.tile([MID, 512], F32, name="ps1t",
                                       tag="ps1t")
                    for wi, (w_t, mov) in enumerate(group1):
                        nc.tensor.matmul(
                            ps1[:], w_t[:], mov(s),
                            start=(wi == 0), stop=(wi == len(group1) - 1))
                    nc.vector.tensor_scalar_add(
                        pre_p[c // 2][:, psl(c, s)], ps1[:], bias1_t[:, 0:1])

            def emit_ps2(t, c):
                ps2 = ps2pool.tile([MID, CHUNK], F32, name="ps2t", tag="ps2t")
                group2 = [(m12_t, lambda s: pre_p[c // 2][:, psl(c, s)])]
                if t > 0:
                    group2.append((m2_t, lambda s: x2_p[c // 2][:, psl(c, s)]))
                for wi, (w_t, mov) in enumerate(group2):
                    for s in range(NSLICE):
                        nc.tensor.matmul(
                            ps2[:, 512 * s:512 * (s + 1)], w_t[:], mov(s),
                            start=(wi == 0), stop=(wi == len(group2) - 1))
                # x2 state: tanh(psum + bias2), written into the pair tile
                nc.scalar.activation(
                    x2_p[c // 2][:, (c % 2) * CHUNK:(c % 2 + 1) * CHUNK],
                    ps2[:], AF.Tanh, bias=bias2_t[:, 0:1])

            for t in range(T):
                if t == 0:
                    xt = xt_first
                else:
                    xt = xpool.tile([FP, BC], F32R, name="xt", tag="xt")
                    for c in range(NCHUNK):
                        nc.sync.dma_start(xt[:, c * CHUNK:(c + 1) * CHUNK],
                                          xT[t][:, c * CHUNK:(c + 1) * CHUNK])
                # pipelined chunk schedule; tanh(x1) runs 2048-wide on the
                # SBUF pre pair once both halves are written
                emit_ps1(t, xt, 0)
                emit_ps1(t, xt, 1)
                emit_ps1(t, xt, 2)
                nc.scalar.activation(x1_p[0][:], pre_p[0][:], AF.Tanh)
                emit_ps2(t, 0)
                emit_ps1(t, xt, 3)
                emit_ps2(t, 1)
                nc.scalar.activation(x1_p[1][:], pre_p[1][:], AF.Tanh)
                emit_ps2(t, 2)
                emit_ps2(t, 3)

            # classifier + final-state DMAs, per pair as soon as the
            # last-step tanh for that pair lands
            out_sb = cpool.tile([10, BC], F32)
            for p in range(NCHUNK // 2):
                c0 = p * PAIR
                nc.sync.dma_start(x1T[:, c0:c0 + PAIR], x1_p[p][:])
                nc.sync.dma_start(x2T[:, c0:c0 + PAIR], x2_p[p][:])
                for c in (2 * p, 2 * p + 1):
                    cc = c * CHUNK
                    for s in range(NSLICE):
                        psf = ps1pool.tile([MID, 512], F32, name="psf",
                                           tag="ps1t")
                        nc.tensor.matmul(
                            psf[0:10, :], fc_w_t[:],
                            x2_p[p][:, psl(c, s)],
                            start=True, stop=True)
                        nc.scalar.activation(
                            out_sb[:, cc + 512 * s:cc + 512 * (s + 1)],
                            psf[0:10, :],
                            AF.Identity, bias=fc_b_t[:, 0:1])
                    nc.sync.dma_start(outT[:, cc:cc + CHUNK],
                                      out_sb[:, cc:cc + CHUNK])

    nc.compile()
    return nc


def kernel(x, binde1, binde2, binde3, binde4,
           w_in, w_res1, w_res12, w_res2, w_res21,
           b_in, b_x1, b_res12, b_x2, b_res21, fc_w, fc_b):
    if "nc" not in _cached:
        _cached["nc"] = _build()
    nc = _cached["nc"]

    x = np.asarray(x, dtype=np.float32)
    xT = np.zeros((T, FP, B), dtype=np.float32)
    xT[:, :F, :] = x.reshape(B, T, F).transpose(1, 2, 0)

    m1 = np.asarray(w_res1 * binde1, dtype=np.float32)
    m21 = np.asarray(w_res21 * binde2, dtype=np.float32)
    m12 = np.asarray(w_res12 * binde3, dtype=np.float32)
    m2 = np.asarray(w_res2 * binde4, dtype=np.float32)
    bias1 = np.asarray(b_in + b_x1 + b_res21, dtype=np.float32).reshape(MID, 1)
    bias2 = np.asarray(b_res12 + b_x2, dtype=np.float32).reshape(MID, 1)
    w_in_np = np.zeros((FP, MID), dtype=np.float32)
    w_in_np[:F] = np.asarray(w_in, dtype=np.float32)
    fc_w_np = np.asarray(fc_w, dtype=np.float32)
    fc_b_np = np.asarray(fc_b, dtype=np.float32).reshape(10, 1)

    shared = dict(w_in=w_in_np, m1=m1, m21=m21, m12=m12, m2=m2,
                  fc_w=fc_w_np, bias1=bias1, bias2=bias2, fc_b=fc_b_np)
    in_maps = []
    for i in range(NCORES):
        xs = np.ascontiguousarray(xT[:, :, i * BC:(i + 1) * BC])
        in_maps.append(dict(xT=xs, **shared))

    # the device occasionally throws a transient unrecoverable-exec-unit
    # error; a fresh run recovers it
    last_err = None
    for attempt in range(3):
        try:
            res = run_bass_kernel_spmd(nc, in_maps, list(range(NCORES)))
            break
        except Exception as e:  # noqa: BLE001
            last_err = e
            time.sleep(2.0)
    else:
        raise last_err

    output = np.empty((B, 10), dtype=np.float32)
    x1 = np.empty((B, MID), dtype=np.float32)
    x2 = np.empty((B, MID), dtype=np.float32)
    for i in range(NCORES):
        sl = slice(i * BC, (i + 1) * BC)
        output[sl] = res.results[i]["outT"].T
        x1[sl] = res.results[i]["x1T"].T
        x2[sl] = res.results[i]["x2T"].T
    return output, x1, x2
